# revision 1
# baseline (speedup 1.0000x reference)
"""ComplEx edge-scoring kernel for Trainium2 (8 NeuronCores, raw Bass).

score[e] = sum_h[ (hr*rr - hi*ri)*tr + (hr*ri + hi*rr)*ti ]
with head/tail rows gathered from z[100000, 256] and rel rows from
rel_emb / rel_emb_imag [50, 128] by edge_type.

Sharding (per the sharding_hint): edges are data-parallel across the 8
cores; z and the rel tables are replicated.  The host packs one gather
source ZF = [z ; concat(rel_emb, rel_emb_imag)] -> [100050, 256] bf16.

Gathers use the fast SWDGE `dma_gather` (CounterMachine descriptor
generation).  Its indices are int16, so ZF is viewed as 4 chunks of
<=32768 rows and edges are bucketed by (head_chunk, tail_chunk); each
bucket's gathers read from fixed chunk base addresses.  Buckets are
dealt round-robin across cores so all 8 cores share one program layout
(SPMD); per-128 padding inside a bucket uses trailing -1 indices, which
dma_gather skips (no DMA traffic).

Per group of <=8 tiles (128 edges each), three dma_gathers (head, tail,
rel - on SWDGE queues 0/1/2, i.e. different Q7 core pairs) land in an
SBUF slot; DVE does the batched complex-rotation elementwise math; the
Scalar engine reduces each tile's 256-wide product row to the score via
activation-accumulate.  NBUF slots keep DMA, DVE and ACT pipelined;
scores accumulate in SBUF and leave in one DMA at the end.  The host
inverts its edge permutation on the way out.
"""

import os

import numpy as np

NUM_NODES = 100000
NUM_RELS = 50
H = 128
TWO_H = 2 * H
N_CORES = 8

P = 128
G = 8  # max tiles per gather group
NBUF = 4  # data buffer slots
CH = 32768  # zf chunk rows (int16 index range)
ZF_ROWS = NUM_NODES + NUM_RELS
NCHUNK = (ZF_ROWS + CH - 1) // CH  # 4
REL_CHUNK = NUM_NODES // CH  # 3
REL_LOCAL = NUM_NODES - REL_CHUNK * CH  # 1696

REDUCE_MODE = os.environ.get("KERNEL_REDUCE", "act")  # "act" | "dve"
DATA_DT = "bfloat16"

_CACHED = {}


def _plan_layout(n_bc):
    """n_bc: per-core edge count per bucket (identical across cores).
    Returns (groups, total_tiles) where each group is a dict with
    bucket, ntiles, nvalid, score_col, idx_col (int16 col offsets)."""
    groups = []
    total_tiles = 0
    idx_col = 0
    for b in range(16):
        n = n_bc[b]
        if n == 0:
            continue
        tiles_b = (n + P - 1) // P
        t0 = 0
        while t0 < tiles_b:
            nt = min(G, tiles_b - t0)
            nvalid = min(n - t0 * P, nt * P)
            groups.append(
                dict(
                    bucket=b,
                    ntiles=nt,
                    nvalid=nvalid,
                    score_col=total_tiles + t0,
                    idx_col=idx_col,
                )
            )
            idx_col += 3 * nt * 8  # 3 sections, nt*128 idxs = nt*8 int16 cols
            t0 += nt
        total_tiles += tiles_b
    return groups, total_tiles, idx_col


def _build_program(groups, total_tiles, idx_cols):
    from concourse import bacc, bass, mybir
    from concourse.library_config import mlp

    ddt = getattr(mybir.dt, DATA_DT)
    nc = bacc.Bacc("TRN2", num_swdge_queues=3)

    zf = nc.dram_tensor("zf", [ZF_ROWS, TWO_H], ddt, kind="ExternalInput")
    offs = nc.dram_tensor("offs", [P, idx_cols], mybir.dt.int16, kind="ExternalInput")
    out = nc.dram_tensor("out", [P, total_tiles], mybir.dt.float32, kind="ExternalOutput")

    FD = TWO_H
    GW = G * FD

    chunks = [zf[c * CH : min((c + 1) * CH, ZF_ROWS)] for c in range(NCHUNK)]

    offs_sb = nc.alloc_sbuf_tensor("offs_sb", [P, idx_cols], mybir.dt.int16)
    data = [nc.alloc_sbuf_tensor(f"data{b}", [P, 3 * GW], ddt) for b in range(NBUF)]
    X = nc.alloc_sbuf_tensor("X", [P, GW], ddt)
    Y0 = nc.alloc_sbuf_tensor("Y0", [P, G * H], ddt)
    Y1 = nc.alloc_sbuf_tensor("Y1", [P, G * H], ddt)
    C = nc.alloc_sbuf_tensor("C", [P, GW], ddt)
    Pm = [nc.alloc_sbuf_tensor(f"Pm{b}", [P, GW], ddt) for b in range(2)]
    scores = nc.alloc_sbuf_tensor("scores", [P, total_tiles], mybir.dt.float32)

    NG = len(groups)

    with (
        nc.Block() as block,
        nc.semaphore("off_sem") as off_sem,
        nc.semaphore("s_sem0") as s_sem0,
        nc.semaphore("s_sem1") as s_sem1,
        nc.semaphore("s_sem2") as s_sem2,
        nc.semaphore("s_sem3") as s_sem3,
        nc.semaphore("pm_sem") as pm_sem,
        nc.semaphore("red_sem") as red_sem,
    ):
        slot_sems = [s_sem0, s_sem1, s_sem2, s_sem3][:NBUF]

        @block.sync
        def _(sync):
            sync.dma_start(out=offs_sb[:], in_=offs[:]).then_inc(off_sem, 16)

        @block.gpsimd
        def _(gpsimd):
            gpsimd.load_library(mlp)
            gpsimd.wait_ge(off_sem, 16)
            nreg = nc.alloc_register(mybir.EngineType.Pool, "nvalid")

            def issue(g):
                grp = groups[g]
                b = g % NBUF
                nt = grp["ntiles"]
                ic = grp["idx_col"]
                S = nt * 8
                sections = (
                    (0, grp["bucket"] // 4),  # head
                    (1, grp["bucket"] % 4),  # tail
                    (2, REL_CHUNK),  # rel
                )
                gpsimd.reg_mov(nreg, grp["nvalid"])
                for s, chunk in sections:
                    gpsimd.dma_gather(
                        data[b][:, s * GW : s * GW + nt * FD].rearrange(
                            "p (j c) -> p j c", c=FD
                        ),
                        chunks[chunk],
                        offs_sb[:, ic + s * S : ic + (s + 1) * S],
                        nt * P,
                        nreg,
                        FD,
                        queue_num=s,
                    ).then_inc(slot_sems[b], 16)

            for g in range(min(NBUF, NG)):
                issue(g)
            for g in range(NG - NBUF):
                gpsimd.wait_ge(pm_sem, g + 1)
                issue(g + NBUF)

        @block.vector
        def _(vector):
            for g in range(NG):
                grp = groups[g]
                b = g % NBUF
                nt = grp["ntiles"]
                d = data[b]
                dv = d[:].rearrange("p (j c) -> p j c", c=FD)
                Hblk = d[:, 0 : nt * FD]
                Hv = dv[:, 0:nt, :]
                Rv = dv[:, 2 * G : 2 * G + nt, :]
                Rblk = d[:, 2 * GW : 2 * GW + nt * FD]
                Xv = X[:].rearrange("p (j c) -> p j c", c=FD)
                Y0v = Y0[:].rearrange("p (j c) -> p j c", c=H)
                Y1v = Y1[:].rearrange("p (j c) -> p j c", c=H)
                Cv = C[:].rearrange("p (j c) -> p j c", c=FD)

                vector.wait_ge(slot_sems[b], 48 * (g // NBUF + 1))
                # X = [hr*rr | hi*ri]
                vector.tensor_tensor(
                    out=X[:, 0 : nt * FD], in0=Hblk, in1=Rblk, op=mybir.AluOpType.mult
                )
                # Y0 = hr*ri ; Y1 = hi*rr
                vector.tensor_tensor(
                    out=Y0v[:, 0:nt, :],
                    in0=Hv[:, :, 0:H],
                    in1=Rv[:, :, H:FD],
                    op=mybir.AluOpType.mult,
                )
                vector.tensor_tensor(
                    out=Y1v[:, 0:nt, :],
                    in0=Hv[:, :, H:FD],
                    in1=Rv[:, :, 0:H],
                    op=mybir.AluOpType.mult,
                )
                # C = [hr*rr - hi*ri | hr*ri + hi*rr]
                vector.tensor_tensor(
                    out=Cv[:, 0:nt, 0:H],
                    in0=Xv[:, 0:nt, 0:H],
                    in1=Xv[:, 0:nt, H:FD],
                    op=mybir.AluOpType.subtract,
                )
                vector.tensor_tensor(
                    out=Cv[:, 0:nt, H:FD],
                    in0=Y0v[:, 0:nt, :],
                    in1=Y1v[:, 0:nt, :],
                    op=mybir.AluOpType.add,
                )
                # Pm = C * tail
                if REDUCE_MODE == "act":
                    if g >= 2:
                        vector.wait_ge(red_sem, g - 1)
                    vector.tensor_tensor(
                        out=Pm[g % 2][:, 0 : nt * FD],
                        in0=C[:, 0 : nt * FD],
                        in1=d[:, GW : GW + nt * FD],
                        op=mybir.AluOpType.mult,
                    ).then_inc(pm_sem, 1)
                else:
                    vector.tensor_tensor(
                        out=Pm[0][:, 0 : nt * FD],
                        in0=C[:, 0 : nt * FD],
                        in1=d[:, GW : GW + nt * FD],
                        op=mybir.AluOpType.mult,
                    )
                    sc = grp["score_col"]
                    vector.tensor_reduce(
                        out=scores[:, sc : sc + nt],
                        in_=Pm[0][:, 0 : nt * FD].rearrange("p (j c) -> p j c", c=FD),
                        axis=mybir.AxisListType.X,
                        op=mybir.AluOpType.add,
                    ).then_inc(pm_sem, 1)

        if REDUCE_MODE == "act":

            @block.scalar
            def _(scalar):
                for g in range(NG):
                    grp = groups[g]
                    nt = grp["ntiles"]
                    scalar.wait_ge(pm_sem, g + 1)
                    pm = Pm[g % 2]
                    for j in range(nt):
                        t = grp["score_col"] + j
                        ins = scalar.activation(
                            out=pm[:, j * FD : (j + 1) * FD],
                            in_=pm[:, j * FD : (j + 1) * FD],
                            func=mybir.ActivationFunctionType.Copy,
                            accum_out=scores[:, t : t + 1],
                        )
                        if j == nt - 1:
                            ins.then_inc(red_sem, 1)

        @block.sync
        def _(sync):
            sync.wait_ge(red_sem if REDUCE_MODE == "act" else pm_sem, NG)
            sync.dma_start(out=out[:], in_=scores[:]).then_inc(off_sem, 16)
            sync.wait_ge(off_sem, 32)

    nc.compile()
    return nc


def _np_dtype():
    if DATA_DT == "bfloat16":
        import ml_dtypes

        return ml_dtypes.bfloat16
    return np.float32


def _prep(z, edge_index, edge_type, rel_emb, rel_emb_imag):
    """Returns (zf, offs_per_core, origpos_per_core, groups, total_tiles, idx_cols)."""
    z = np.asarray(z, dtype=np.float32)
    rel2 = np.concatenate(
        [np.asarray(rel_emb, np.float32), np.asarray(rel_emb_imag, np.float32)],
        axis=1,
    )
    zf = np.ascontiguousarray(np.concatenate([z, rel2], axis=0).astype(_np_dtype()))

    E = edge_index.shape[1]
    head = np.asarray(edge_index[0], np.int64)
    tail = np.asarray(edge_index[1], np.int64)
    rel = np.asarray(edge_type, np.int64) + NUM_NODES

    bucket = (head // CH) * 4 + tail // CH
    order = np.argsort(bucket, kind="stable")
    counts = np.bincount(bucket, minlength=16)

    # pad each global bucket to a multiple of N_CORES with dummy edges
    hb, tb, rb, ob = [], [], [], []
    for b in range(16):
        sel = order[counts[:b].sum() : counts[: b + 1].sum()]
        pad = (-len(sel)) % N_CORES
        hb.append(np.concatenate([head[sel], np.full(pad, (b // 4) * CH)]))
        tb.append(np.concatenate([tail[sel], np.full(pad, (b % 4) * CH)]))
        rb.append(np.concatenate([rel[sel], np.full(pad, NUM_NODES)]))
        ob.append(np.concatenate([sel, np.full(pad, -1)]))
    n_bc = [len(x) // N_CORES for x in hb]

    groups, total_tiles, idx_cols = _plan_layout(n_bc)

    offs_all, orig_all = [], []
    for c in range(N_CORES):
        idx16 = np.full((16, idx_cols), -1, np.int16)
        origpos = np.full(total_tiles * P, -1, np.int64)
        for g in groups:
            b = g["bucket"]
            nt, ic = g["ntiles"], g["idx_col"]
            t0 = (g["score_col"] - _bucket_tile0(groups, b)) * P  # edge offset in bucket
            lo = t0
            hi = min(t0 + nt * P, n_bc[b])
            sel = slice(lo, hi)
            hloc = hb[b][c::N_CORES][sel] - (b // 4) * CH
            tloc = tb[b][c::N_CORES][sel] - (b % 4) * CH
            rloc = rb[b][c::N_CORES][sel] - REL_CHUNK * CH
            op = ob[b][c::N_CORES][sel]
            n = hi - lo
            S = nt * 8
            for s, loc in ((0, hloc), (1, tloc), (2, rloc)):
                a = np.full(nt * P, -1, np.int16)
                a[:n] = loc.astype(np.int16)
                idx16[:, ic + s * S : ic + (s + 1) * S] = a.reshape(S, 16).T
            sc = g["score_col"]
            orig_seg = np.full(nt * P, -1, np.int64)
            orig_seg[:n] = op
            origpos[sc * P : sc * P + nt * P] = orig_seg
        offs_all.append(np.ascontiguousarray(np.tile(idx16, (8, 1))))
        orig_all.append(origpos)
    return zf, offs_all, orig_all, groups, total_tiles, idx_cols


def _bucket_tile0(groups, b):
    for g in groups:
        if g["bucket"] == b:
            return g["score_col"]
    return 0


class Runner:
    def __init__(self, nc, n_cores=8):
        import jax
        from jax.sharding import Mesh, PartitionSpec
        from jax.experimental.shard_map import shard_map
        from concourse import mybir
        from concourse.bass2jax import (
            _bass_exec_p,
            install_neuronx_cc_hook,
            partition_id_tensor,
        )

        install_neuronx_cc_hook()
        self.n_cores = n_cores

        partition_name = nc.partition_id_tensor.name if nc.partition_id_tensor else None
        in_names, out_names, out_avals = [], [], []
        for alloc in nc.m.functions[0].allocations:
            if not isinstance(alloc, mybir.MemoryLocationSet):
                continue
            name = alloc.memorylocations[0].name
            if alloc.kind == "ExternalInput":
                if name != partition_name:
                    in_names.append(name)
            elif alloc.kind == "ExternalOutput":
                out_names.append(name)
                out_avals.append(
                    jax.core.ShapedArray(
                        tuple(alloc.tensor_shape), mybir.dt.np(alloc.dtype)
                    )
                )
        self.in_names = list(in_names)
        self.out_names = out_names
        self.out_avals = out_avals
        bind_in_names = tuple(in_names + out_names)
        if partition_name is not None:
            bind_in_names = bind_in_names + (partition_name,)

        def _body(*args):
            operands = list(args)
            if partition_name is not None:
                operands.append(partition_id_tensor())
            outs = _bass_exec_p.bind(
                *operands,
                out_avals=tuple(out_avals),
                in_names=bind_in_names,
                out_names=tuple(out_names),
                lowering_input_output_aliases=(),
                sim_require_finite=True,
                sim_require_nnan=True,
                nc=nc,
            )
            return tuple(outs)

        devices = jax.devices()[:n_cores]
        mesh = Mesh(np.asarray(devices), ("core",))
        self._mesh = mesh
        n_args = len(in_names) + len(out_names)
        self._fn = jax.jit(
            shard_map(
                _body,
                mesh=mesh,
                in_specs=(PartitionSpec("core"),) * n_args,
                out_specs=(PartitionSpec("core"),) * len(out_names),
                check_rep=False,
            ),
            keep_unused=True,
        )

    def put(self, per_core_inputs):
        """per_core_inputs: dict name -> list of per-core np arrays (or one
        array replicated)."""
        import jax
        from jax.sharding import NamedSharding, PartitionSpec

        sh = NamedSharding(self._mesh, PartitionSpec("core"))
        args = []
        for name in self.in_names:
            arrs = per_core_inputs[name]
            if isinstance(arrs, np.ndarray):
                arrs = [arrs] * self.n_cores
            args.append(np.concatenate([np.asarray(a) for a in arrs], axis=0))
        for av in self.out_avals:
            args.append(
                np.zeros((self.n_cores * av.shape[0], *av.shape[1:]), av.dtype)
            )
        self.args = [jax.device_put(a, sh) for a in args]
        for a in self.args:
            a.block_until_ready()

    def run(self):
        outs = self._fn(*self.args)
        outs = [np.asarray(o) for o in outs]
        return {
            name: [
                o.reshape(self.n_cores, *self.out_avals[i].shape)[c]
                for c in range(self.n_cores)
            ]
            for i, (name, o) in enumerate(zip(self.out_names, outs))
        }

    def bench(self, iters=10):
        import time

        self._fn(*self.args)[0].block_until_ready()
        times = []
        for _ in range(iters):
            t0 = time.perf_counter()
            self._fn(*self.args)[0].block_until_ready()
            times.append(time.perf_counter() - t0)
        return min(times), times


def kernel(z, edge_index, edge_type, rel_emb, rel_emb_imag):
    zf, offs_all, orig_all, groups, total_tiles, idx_cols = _prep(
        z, edge_index, edge_type, rel_emb, rel_emb_imag
    )

    key = (tuple((g["bucket"], g["ntiles"], g["nvalid"]) for g in groups), REDUCE_MODE)
    if _CACHED.get("key") != key:
        nc = _build_program(groups, total_tiles, idx_cols)
        _CACHED["runner"] = Runner(nc, n_cores=N_CORES)
        _CACHED["key"] = key
    runner = _CACHED["runner"]

    runner.put({"zf": zf, "offs": offs_all})
    res = runner.run()

    if int(os.environ.get("KERNEL_BENCH", "0")):
        best, times = runner.bench(iters=int(os.environ.get("KERNEL_BENCH_ITERS", "10")))
        _CACHED["exec_time_ns"] = best * 1e9
        _CACHED["bench_times"] = times

    E = edge_index.shape[1]
    result = np.zeros(E, np.float32)
    for c in range(N_CORES):
        o = res["out"][c]  # [128, total_tiles]
        flat = o.T.ravel()  # slot k = tile*128+p
        op = orig_all[c]
        valid = op >= 0
        result[op[valid]] = flat[valid]
    return result



# revision 11
# speedup vs baseline: 85.0772x; 85.0772x over previous
"""ComplEx edge-scoring kernel for Trainium2 (8 NeuronCores, raw Bass).

score[e] = sum_h[ (hr*rr - hi*ri)*tr + (hr*ri + hi*rr)*ti ]
with head/tail rows gathered from z[100000, 256] and rel rows from
rel_emb / rel_emb_imag [50, 128] by edge_type.

Sharding (per the sharding_hint): edges are data-parallel across the 8
cores; z and the rel tables are replicated.  The host packs one gather
source ZF = [z ; concat(rel_emb, rel_emb_imag)] -> [100050, 256] bf16.

Gathers use the fast SWDGE `dma_gather` (CounterMachine descriptor
generation).  Its indices are int16, so ZF is viewed as 4 chunks of
<=32768 rows and edges are bucketed by (head_chunk, tail_chunk); each
bucket's gathers read from fixed chunk base addresses.  Buckets are
dealt round-robin across cores so all 8 cores share one program layout
(SPMD); per-128 padding inside a bucket uses trailing -1 indices, which
dma_gather skips (no DMA traffic).

Per group of <=G tiles (128 edges each), three dma_gathers (head, tail,
rel - rotating over the SWDGE queues, i.e. different Q7 core pairs) land
in an SBUF slot; DVE does the batched complex-rotation elementwise math;
the Scalar engine reduces each tile's 256-wide product row to the score
via activation-accumulate.  NBUF slots keep DMA, DVE and ACT pipelined;
scores accumulate in SBUF and leave in one DMA at the end.  The host
inverts its edge permutation on the way out.

The SWDGE descriptor carveout (dynamic_dma_scratch_size) is sized to
G*128 descriptors so one gather instruction can cover a whole group.
"""

import os

import numpy as np

NUM_NODES = 100000
NUM_RELS = 50
H = 128
TWO_H = 2 * H
N_CORES = 8

P = 128
G = int(os.environ.get("KERNEL_G", "8"))  # max tiles per gather group
NBUF = int(os.environ.get("KERNEL_NBUF", "6"))  # data buffer slots
NQ = int(os.environ.get("KERNEL_NQ", "4"))  # SWDGE queues
CH = 32768  # zf chunk rows (int16 index range)
ZF_ROWS = NUM_NODES + NUM_RELS
NCHUNK = (ZF_ROWS + CH - 1) // CH  # 4
REL_CHUNK = NUM_NODES // CH  # 3
REL_LOCAL = NUM_NODES - REL_CHUNK * CH  # 1696

REDUCE_MODE = os.environ.get("KERNEL_REDUCE", "act")  # "ttr" | "act" | "dve"
DATA_DT = "bfloat16"

_CACHED = {}


def _plan_layout(n_bc):
    """n_bc: per-core edge count per bucket (identical across cores).
    Returns (groups, total_tiles) where each group is a dict with
    bucket, ntiles, nvalid, score_col, idx_col (int16 col offsets)."""
    groups = []
    total_tiles = 0
    idx_col = 0
    for b in range(16):
        n = n_bc[b]
        if n == 0:
            continue
        tiles_b = (n + P - 1) // P
        t0 = 0
        while t0 < tiles_b:
            nt = min(G, tiles_b - t0)
            nvalid = min(n - t0 * P, nt * P)
            groups.append(
                dict(
                    bucket=b,
                    ntiles=nt,
                    nvalid=nvalid,
                    score_col=total_tiles + t0,
                    idx_col=idx_col,
                )
            )
            idx_col += 3 * nt * 8  # 3 sections, nt*128 idxs = nt*8 int16 cols
            t0 += nt
        total_tiles += tiles_b
    return groups, total_tiles, idx_col


def _build_program(groups, total_tiles, idx_cols):
    from concourse import bacc, bass, mybir
    from concourse.library_config import mlp

    ddt = getattr(mybir.dt, DATA_DT)
    # SWDGE descriptor carveout: G*128 descs/gather x 16B must fit
    nc = bacc.Bacc(
        "TRN2",
        num_swdge_queues=NQ,
        dynamic_dma_scratch_size=max(16384, G * 128 * 16),
    )

    zf = nc.dram_tensor("zf", [ZF_ROWS, TWO_H], ddt, kind="ExternalInput")
    offs = nc.dram_tensor("offs", [P, idx_cols], mybir.dt.int16, kind="ExternalInput")
    out = nc.dram_tensor("out", [P, total_tiles], mybir.dt.float32, kind="ExternalOutput")

    FD = TWO_H
    GW = G * FD

    chunks = [zf[c * CH : min((c + 1) * CH, ZF_ROWS)] for c in range(NCHUNK)]

    offs_sb = nc.alloc_sbuf_tensor("offs_sb", [P, idx_cols], mybir.dt.int16)
    data = [nc.alloc_sbuf_tensor(f"data{b}", [P, 3 * GW], ddt) for b in range(NBUF)]
    X = nc.alloc_sbuf_tensor("X", [P, GW], ddt)
    Y0 = nc.alloc_sbuf_tensor("Y0", [P, G * H], ddt)
    Y1 = nc.alloc_sbuf_tensor("Y1", [P, G * H], ddt)
    C = nc.alloc_sbuf_tensor("C", [P, GW], ddt)
    Pm = [nc.alloc_sbuf_tensor(f"Pm{b}", [P, GW], ddt) for b in range(2)]
    scores = nc.alloc_sbuf_tensor("scores", [P, total_tiles], mybir.dt.float32)

    NG = len(groups)

    import contextlib

    with (
        contextlib.ExitStack() as stack,
        nc.Block() as block,
        nc.semaphore("off_sem") as off_sem,
        nc.semaphore("pm_sem") as pm_sem,
        nc.semaphore("red_sem") as red_sem,
    ):
        slot_sems = [
            stack.enter_context(nc.semaphore(f"s_sem{i}")) for i in range(NBUF)
        ]

        @block.sync
        def _(sync):
            sync.dma_start(out=offs_sb[:], in_=offs[:]).then_inc(off_sem, 16)

        @block.gpsimd
        def _(gpsimd):
            gpsimd.load_library(mlp)
            gpsimd.wait_ge(off_sem, 16)
            nreg = nc.alloc_register(mybir.EngineType.Pool, "nvalid")

            def issue(g):
                grp = groups[g]
                b = g % NBUF
                nt = grp["ntiles"]
                ic = grp["idx_col"]
                S = nt * 8
                sections = (
                    (0, grp["bucket"] // 4),  # head
                    (1, grp["bucket"] % 4),  # tail
                    (2, REL_CHUNK),  # rel
                )
                gpsimd.reg_mov(nreg, grp["nvalid"])
                for s, chunk in sections:
                    gpsimd.dma_gather(
                        data[b][:, s * GW : s * GW + nt * FD].rearrange(
                            "p (j c) -> p j c", c=FD
                        ),
                        chunks[chunk],
                        offs_sb[:, ic + s * S : ic + (s + 1) * S],
                        nt * P,
                        nreg,
                        FD,
                        queue_num=(g * 3 + s) % NQ,
                    ).then_inc(slot_sems[b], 16)

            for g in range(min(NBUF, NG)):
                issue(g)
            for g in range(NG - NBUF):
                gpsimd.wait_ge(pm_sem, g + 1)
                issue(g + NBUF)

        @block.vector
        def _(vector):
            for g in range(NG):
                grp = groups[g]
                b = g % NBUF
                nt = grp["ntiles"]
                d = data[b]
                dv = d[:].rearrange("p (j c) -> p j c", c=FD)
                Hblk = d[:, 0 : nt * FD]
                Hv = dv[:, 0:nt, :]
                Rv = dv[:, 2 * G : 2 * G + nt, :]
                Rblk = d[:, 2 * GW : 2 * GW + nt * FD]
                Xv = X[:].rearrange("p (j c) -> p j c", c=FD)
                Y0v = Y0[:].rearrange("p (j c) -> p j c", c=H)
                Y1v = Y1[:].rearrange("p (j c) -> p j c", c=H)
                Cv = C[:].rearrange("p (j c) -> p j c", c=FD)

                vector.wait_ge(slot_sems[b], 48 * (g // NBUF + 1))
                # X = [hr*rr | hi*ri]
                vector.tensor_tensor(
                    out=X[:, 0 : nt * FD], in0=Hblk, in1=Rblk, op=mybir.AluOpType.mult
                )
                # Y0 = hr*ri ; Y1 = hi*rr
                vector.tensor_tensor(
                    out=Y0v[:, 0:nt, :],
                    in0=Hv[:, :, 0:H],
                    in1=Rv[:, :, H:FD],
                    op=mybir.AluOpType.mult,
                )
                vector.tensor_tensor(
                    out=Y1v[:, 0:nt, :],
                    in0=Hv[:, :, H:FD],
                    in1=Rv[:, :, 0:H],
                    op=mybir.AluOpType.mult,
                )
                # C = [hr*rr - hi*ri | hr*ri + hi*rr]
                vector.tensor_tensor(
                    out=Cv[:, 0:nt, 0:H],
                    in0=Xv[:, 0:nt, 0:H],
                    in1=Xv[:, 0:nt, H:FD],
                    op=mybir.AluOpType.subtract,
                )
                vector.tensor_tensor(
                    out=Cv[:, 0:nt, H:FD],
                    in0=Y0v[:, 0:nt, :],
                    in1=Y1v[:, 0:nt, :],
                    op=mybir.AluOpType.add,
                )
                # Pm = C * tail
                if REDUCE_MODE == "ttr":
                    # fused multiply + row-reduce per tile on DVE; scores
                    # written directly, Scalar engine unused
                    sc = grp["score_col"]
                    for j in range(nt):
                        ins = vector.tensor_tensor_reduce(
                            out=Pm[0][:, j * FD : (j + 1) * FD],
                            in0=C[:, j * FD : (j + 1) * FD],
                            in1=d[:, GW + j * FD : GW + (j + 1) * FD],
                            scale=1.0,
                            scalar=0.0,
                            op0=mybir.AluOpType.mult,
                            op1=mybir.AluOpType.add,
                            accum_out=scores[:, sc + j : sc + j + 1],
                        )
                        if j == nt - 1:
                            ins.then_inc(pm_sem, 1)
                elif REDUCE_MODE == "act":
                    if g >= 2:
                        vector.wait_ge(red_sem, g - 1)
                    vector.tensor_tensor(
                        out=Pm[g % 2][:, 0 : nt * FD],
                        in0=C[:, 0 : nt * FD],
                        in1=d[:, GW : GW + nt * FD],
                        op=mybir.AluOpType.mult,
                    ).then_inc(pm_sem, 1)
                else:
                    vector.tensor_tensor(
                        out=Pm[0][:, 0 : nt * FD],
                        in0=C[:, 0 : nt * FD],
                        in1=d[:, GW : GW + nt * FD],
                        op=mybir.AluOpType.mult,
                    )
                    sc = grp["score_col"]
                    vector.tensor_reduce(
                        out=scores[:, sc : sc + nt],
                        in_=Pm[0][:, 0 : nt * FD].rearrange("p (j c) -> p j c", c=FD),
                        axis=mybir.AxisListType.X,
                        op=mybir.AluOpType.add,
                    ).then_inc(pm_sem, 1)

        if REDUCE_MODE == "act":

            @block.scalar
            def _(scalar):
                for g in range(NG):
                    grp = groups[g]
                    nt = grp["ntiles"]
                    scalar.wait_ge(pm_sem, g + 1)
                    pm = Pm[g % 2]
                    for j in range(nt):
                        t = grp["score_col"] + j
                        ins = scalar.activation(
                            out=pm[:, j * FD : (j + 1) * FD],
                            in_=pm[:, j * FD : (j + 1) * FD],
                            func=mybir.ActivationFunctionType.Copy,
                            accum_out=scores[:, t : t + 1],
                        )
                        if j == nt - 1:
                            ins.then_inc(red_sem, 1)

        @block.sync
        def _(sync):
            sync.wait_ge(red_sem if REDUCE_MODE == "act" else pm_sem, NG)
            sync.dma_start(out=out[:], in_=scores[:]).then_inc(off_sem, 16)
            sync.wait_ge(off_sem, 32)

    nc.compile()
    return nc


def _np_dtype():
    if DATA_DT == "bfloat16":
        import ml_dtypes

        return ml_dtypes.bfloat16
    return np.float32


def _prep(z, edge_index, edge_type, rel_emb, rel_emb_imag):
    """Returns (zf, offs_per_core, origpos_per_core, groups, total_tiles, idx_cols)."""
    z = np.asarray(z, dtype=np.float32)
    rel2 = np.concatenate(
        [np.asarray(rel_emb, np.float32), np.asarray(rel_emb_imag, np.float32)],
        axis=1,
    )
    zf = np.ascontiguousarray(np.concatenate([z, rel2], axis=0).astype(_np_dtype()))

    E = edge_index.shape[1]
    head = np.asarray(edge_index[0], np.int64)
    tail = np.asarray(edge_index[1], np.int64)
    rel = np.asarray(edge_type, np.int64) + NUM_NODES

    bucket = (head // CH) * 4 + tail // CH
    order = np.argsort(bucket, kind="stable")
    counts = np.bincount(bucket, minlength=16)

    # pad each global bucket to a multiple of N_CORES with dummy edges
    hb, tb, rb, ob = [], [], [], []
    for b in range(16):
        sel = order[counts[:b].sum() : counts[: b + 1].sum()]
        pad = (-len(sel)) % N_CORES
        hb.append(np.concatenate([head[sel], np.full(pad, (b // 4) * CH)]))
        tb.append(np.concatenate([tail[sel], np.full(pad, (b % 4) * CH)]))
        rb.append(np.concatenate([rel[sel], np.full(pad, NUM_NODES)]))
        ob.append(np.concatenate([sel, np.full(pad, -1)]))
    n_bc = [len(x) // N_CORES for x in hb]

    groups, total_tiles, idx_cols = _plan_layout(n_bc)

    offs_all, orig_all = [], []
    for c in range(N_CORES):
        idx16 = np.full((16, idx_cols), -1, np.int16)
        origpos = np.full(total_tiles * P, -1, np.int64)
        for g in groups:
            b = g["bucket"]
            nt, ic = g["ntiles"], g["idx_col"]
            t0 = (g["score_col"] - _bucket_tile0(groups, b)) * P  # edge offset in bucket
            lo = t0
            hi = min(t0 + nt * P, n_bc[b])
            sel = slice(lo, hi)
            hloc = hb[b][c::N_CORES][sel] - (b // 4) * CH
            tloc = tb[b][c::N_CORES][sel] - (b % 4) * CH
            rloc = rb[b][c::N_CORES][sel] - REL_CHUNK * CH
            op = ob[b][c::N_CORES][sel]
            n = hi - lo
            S = nt * 8
            for s, loc in ((0, hloc), (1, tloc), (2, rloc)):
                a = np.full(nt * P, -1, np.int16)
                a[:n] = loc.astype(np.int16)
                idx16[:, ic + s * S : ic + (s + 1) * S] = a.reshape(S, 16).T
            sc = g["score_col"]
            orig_seg = np.full(nt * P, -1, np.int64)
            orig_seg[:n] = op
            origpos[sc * P : sc * P + nt * P] = orig_seg
        offs_all.append(np.ascontiguousarray(np.tile(idx16, (8, 1))))
        orig_all.append(origpos)
    return zf, offs_all, orig_all, groups, total_tiles, idx_cols


def _bucket_tile0(groups, b):
    for g in groups:
        if g["bucket"] == b:
            return g["score_col"]
    return 0


class Runner:
    def __init__(self, nc, n_cores=8):
        import jax
        from jax.sharding import Mesh, PartitionSpec
        from jax.experimental.shard_map import shard_map
        from concourse import mybir
        from concourse.bass2jax import (
            _bass_exec_p,
            install_neuronx_cc_hook,
            partition_id_tensor,
        )

        install_neuronx_cc_hook()
        self.n_cores = n_cores

        partition_name = nc.partition_id_tensor.name if nc.partition_id_tensor else None
        in_names, out_names, out_avals = [], [], []
        for alloc in nc.m.functions[0].allocations:
            if not isinstance(alloc, mybir.MemoryLocationSet):
                continue
            name = alloc.memorylocations[0].name
            if alloc.kind == "ExternalInput":
                if name != partition_name:
                    in_names.append(name)
            elif alloc.kind == "ExternalOutput":
                out_names.append(name)
                out_avals.append(
                    jax.core.ShapedArray(
                        tuple(alloc.tensor_shape), mybir.dt.np(alloc.dtype)
                    )
                )
        self.in_names = list(in_names)
        self.out_names = out_names
        self.out_avals = out_avals
        bind_in_names = tuple(in_names + out_names)
        if partition_name is not None:
            bind_in_names = bind_in_names + (partition_name,)

        def _body(*args):
            operands = list(args)
            if partition_name is not None:
                operands.append(partition_id_tensor())
            outs = _bass_exec_p.bind(
                *operands,
                out_avals=tuple(out_avals),
                in_names=bind_in_names,
                out_names=tuple(out_names),
                lowering_input_output_aliases=(),
                sim_require_finite=True,
                sim_require_nnan=True,
                nc=nc,
            )
            return tuple(outs)

        devices = jax.devices()[:n_cores]
        mesh = Mesh(np.asarray(devices), ("core",))
        self._mesh = mesh
        n_args = len(in_names) + len(out_names)
        self._fn = jax.jit(
            shard_map(
                _body,
                mesh=mesh,
                in_specs=(PartitionSpec("core"),) * n_args,
                out_specs=(PartitionSpec("core"),) * len(out_names),
                check_rep=False,
            ),
            keep_unused=True,
        )

    def put(self, per_core_inputs):
        """per_core_inputs: dict name -> list of per-core np arrays (or one
        array replicated)."""
        import jax
        from jax.sharding import NamedSharding, PartitionSpec

        sh = NamedSharding(self._mesh, PartitionSpec("core"))
        args = []
        for name in self.in_names:
            arrs = per_core_inputs[name]
            if isinstance(arrs, np.ndarray):
                arrs = [arrs] * self.n_cores
            args.append(np.concatenate([np.asarray(a) for a in arrs], axis=0))
        for av in self.out_avals:
            args.append(
                np.zeros((self.n_cores * av.shape[0], *av.shape[1:]), av.dtype)
            )
        self.args = [jax.device_put(a, sh) for a in args]
        for a in self.args:
            a.block_until_ready()

    def run(self):
        outs = self._fn(*self.args)
        outs = [np.asarray(o) for o in outs]
        return {
            name: [
                o.reshape(self.n_cores, *self.out_avals[i].shape)[c]
                for c in range(self.n_cores)
            ]
            for i, (name, o) in enumerate(zip(self.out_names, outs))
        }

    def bench(self, iters=10):
        import time

        self._fn(*self.args)[0].block_until_ready()
        times = []
        for _ in range(iters):
            t0 = time.perf_counter()
            self._fn(*self.args)[0].block_until_ready()
            times.append(time.perf_counter() - t0)
        return min(times), times


def _unshard(out_per_core, orig_all, E):
    result = np.zeros(E, np.float32)
    for c in range(N_CORES):
        o = out_per_core[c]  # [128, total_tiles]
        flat = o.T.ravel()  # slot k = tile*128+p
        op = orig_all[c]
        valid = op >= 0
        result[op[valid]] = flat[valid]
    return result


def kernel(z, edge_index, edge_type, rel_emb, rel_emb_imag):
    zf, offs_all, orig_all, groups, total_tiles, idx_cols = _prep(
        z, edge_index, edge_type, rel_emb, rel_emb_imag
    )
    E = edge_index.shape[1]
    key = (tuple((g["bucket"], g["ntiles"], g["nvalid"]) for g in groups), REDUCE_MODE)

    if int(os.environ.get("KERNEL_PROFILE", "0")):
        # test-only path: execute once via run_bass_kernel_spmd with NTFF
        # tracing; exec_time_ns is the on-device execution span.
        import tempfile

        from concourse import bass_utils

        if _CACHED.get("key") != key:
            _CACHED["nc"] = _build_program(groups, total_tiles, idx_cols)
            _CACHED["key"] = key
            _CACHED.pop("runner", None)
        in_maps = [{"zf": zf, "offs": offs_all[c]} for c in range(N_CORES)]
        res = bass_utils.run_bass_kernel_spmd(
            _CACHED["nc"],
            in_maps,
            core_ids=list(range(N_CORES)),
            trace=True,
            tmpdir=tempfile.mkdtemp(prefix="kernel_ntff_"),
        )
        _CACHED["exec_time_ns"] = res.exec_time_ns
        return _unshard([r["out"] for r in res.results], orig_all, E)

    if _CACHED.get("key") != key or "runner" not in _CACHED:
        nc = _build_program(groups, total_tiles, idx_cols)
        _CACHED["runner"] = Runner(nc, n_cores=N_CORES)
        _CACHED["key"] = key
    runner = _CACHED["runner"]

    runner.put({"zf": zf, "offs": offs_all})
    res = runner.run()

    if int(os.environ.get("KERNEL_BENCH", "0")):
        best, times = runner.bench(iters=int(os.environ.get("KERNEL_BENCH_ITERS", "10")))
        _CACHED["exec_time_ns"] = best * 1e9
        _CACHED["bench_times"] = times

    return _unshard(res["out"], orig_all, E)

